# revision 49
# baseline (speedup 1.0000x reference)
"""AdaPT quantized linear (int8-exact via fp16 matmul) on 8 TRN2 NeuronCores.

Reference computes:
    qx = clip(round(x * 127/amax_x), -127, 127)        [N, K] int8
    qw = clip(round(w * 127/amax_w), -127, 127)        [M, K] int8
    out = (qx @ qw.T) / ((127/amax_x)*(127/amax_w)) + bias

Strategy: data-parallel over the 8192-token dim (1024 tokens/core), full
weight on every core, no collectives.  All int8 values are exactly
representable in fp16, the PE multiplies fp16 exactly (products < 2^14)
and accumulates in fp32 (partial sums << 2^24), so the fp16 matmul
reproduces the int8 systolic GEMM bit-exactly.

Rounding trick: fp16 has unit spacing on [1024, 2048), so converting
(x*scale + 1536) from fp32 to fp16 rounds the fractional part to the
nearest integer with ties-to-even -- exactly jnp.round.  The quant chain
is then: affine convert to fp16 (ScalarE Copy with immediate bias, or DVE
mult+add dual-op), then fp16 min/max clip and subtract on DVE, which runs
16-bit SBUF-to-SBUF ops at 4x rate.

Engine plan (per core):
  - PE: 2048 accumulating fp16 matmuls (32 m-tiles x 32 k-tiles x 2
    banks of 512 tokens) -- the 437us roofline everything else hides
    under.  A short junk-matmul burst covers the pstate ramp; junk is
    also interleaved into the first wave so the PE never idles (idle
    resets the clock ramp).  (fp8 DoubleRow was measured at 2x per
    k-tile on HW, so any exact 3-product int8 split is 1.5x slower
    than this single fp16 pass -- not viable.)
  - ScalarE: x affine converts (half the pairs), prologue w affine
    converts (fp32 1x ops moved off the loaded DVE), PSUM dequant+bias,
    and the output-store DMA issues.
  - DVE: everything fp16 (clips/subtracts at 4x), steady-state w
    affines, x affine for the other half of the pairs.
  - SP(sync): input DMA issues only.  dma_start blocks its issuing
    engine in program order, so weight DMAs are issued 2 m-tiles ahead
    and never behind dependent work.

The prologue is DMA-bandwidth-bound, not engine-bound: x's 16.8MB plus
the first ~6 weight tiles saturate the DMA for the first ~70us, which
caps how early the 4 PSUM-open m-tiles can retire (structural deficit
~15us).  The wave schedule (x-priority issue order, half-token x
pieces in the first waves, fine w sub-chunks with a full wave of
slack) sits essentially at that bound.  bias is pre-transposed to
[P, MT] on the host: the naive [M]->(p,o) gather DMA stalled its
issuing queue ~11us right when the x-pair issues mattered.  The last
m-tile runs its two banks serially so only one bank's dequant+store
remains after the final matmul (~2us tail).

Output is stored as fp16 ([M, tokens/core] transposed per core, so the
bias rides the ScalarE per-partition bias port) and upcast on the host;
the ~3e-4 relative rounding is far inside the 2e-2 gate.
"""

import sys

import numpy as np

sys.path.insert(0, "/opt/trn_rl_repo")

N, K, M = 8192, 4096, 4096
N_CORES = 8
TPC = N // N_CORES  # tokens per core
P = 128
KT = K // P   # 32 k-tiles
MT = M // P   # 32 m-tiles
TF = 512      # matmul moving free dim (one PSUM bank of fp32)
NTF = TPC // TF
XCH = 4       # x chunks (waves); 4 pairs (8 k-tiles) each
NPAIR = KT // 2
PPC = NPAIR // XCH
MAGIC = 1536.0   # fp16 unit-spacing window base: round via fp32->fp16 convert
MAXV = 127.0
HI = MAGIC + MAXV
LO = MAGIC - MAXV
JUNK_PRE = 14    # PE pstate-ramp warmup matmuls before first real work
JUNK_WAVE = 2    # junk interleaved between wave-0 pair groups


def build(s_x: float, s_w: float, inv_s: float):
    import concourse.mybir as mybir
    import concourse.tile as tile
    from concourse import bacc

    dt = mybir.dt
    AF = mybir.ActivationFunctionType
    OP = mybir.AluOpType

    nc = bacc.Bacc("TRN2", target_bir_lowering=False, debug=False,
                   num_devices=N_CORES)

    xt = nc.declare_dram_parameter("xt", [K, TPC], dt.float32, isOutput=False)
    wt = nc.declare_dram_parameter("wt", [MT, K, P], dt.float32, isOutput=False)
    # bias pre-transposed to [P, MT] on the host: the [M]->(p,o) gather
    # was a 4096x4B strided DMA that blocked the Sync queue ~11us right
    # when the x-pair DMA issues mattered most
    bias = nc.declare_dram_parameter("bias", [P, MT], dt.float32,
                                     isOutput=False)
    out = nc.declare_dram_parameter("out", [M, TPC], dt.float16, isOutput=True)

    with tile.TileContext(nc) as tc:
        with (
            tc.tile_pool(name="xq", bufs=1) as xq_pool,
            tc.tile_pool(name="xs", bufs=6) as xs_pool,
            tc.tile_pool(name="wsf", bufs=4) as wsf_pool,
            tc.tile_pool(name="wsb", bufs=2) as wsb_pool,
            tc.tile_pool(name="wq", bufs=4) as wq_pool,
            tc.tile_pool(name="cst", bufs=1) as cst_pool,
            tc.tile_pool(name="outp", bufs=6) as out_pool,
            tc.tile_pool(name="ps", bufs=4, space="PSUM") as psum_pool,
            tc.tile_pool(name="junk", bufs=1) as junk_pool,
        ):
            # ---- bookkeeping helpers ---------------------------------
            # Junk buffer lives OUTSIDE tile tracking: the first warmup
            # matmuls read garbage (their start=True target is reset by
            # the first real matmul anyway) without waiting on the memset.
            # The memset zeroes it long before any start=False junk runs,
            # so post-start junk accumulates exactly 0 into live banks.
            junk_t = nc.alloc_sbuf_tensor("junk_sb", [P, TF], dt.float16)
            junk_sb = junk_t.ap()

            def junk_mms(n, tgt, live=False):
                # live=True: the target bank's accumulation group is open;
                # zero-junk rides along with start=False (adds 0.0).
                for _ in range(n):
                    nc.tensor.matmul(tgt[:], junk_sb[:, :P], junk_sb[:],
                                     start=not live, stop=False,
                                     skip_group_check=True)

            xq_tiles = [
                xq_pool.tile([P, 2, TPC], dt.float16, name=f"xq{p}",
                             tag=f"xq{p}")
                for p in range(NPAIR)
            ]

            def quant_chain(dst, src, scale, aff_scalar):
                # dst fp16 = clip(round(src*scale), -127, 127) + 1536,
                # recentred; the affine's fp32->fp16 convert is the round.
                if aff_scalar:
                    nc.scalar.activation(dst, src, AF.Copy,
                                         bias=MAGIC, scale=scale)
                else:
                    nc.vector.tensor_scalar(dst, src, scale, MAGIC,
                                            OP.mult, OP.add)
                nc.vector.tensor_scalar(dst, dst, HI, LO, OP.min, OP.max)
                nc.vector.tensor_scalar(dst, dst, MAGIC, None, OP.subtract)

            def issue_x_pair(kp, fine=False):
                # per-k-tile DMAs (0.5MB arrival quantum).  fine=True
                # (DMA-bound early waves): split each k-tile into token
                # halves so the bank-0 matmuls can start ~1.5us before
                # the k-tile's second half lands.
                kt0 = 2 * kp
                xs = xs_pool.tile([P, 2, TPC], dt.float32, name="xs")
                pieces = ((0, TF), (TF, TPC)) if fine else ((0, TPC),)
                for h in range(2):
                    for lo, hi in pieces:
                        nc.sync.dma_start(
                            xs[:, h, lo:hi],
                            xt[(kt0 + h) * P:(kt0 + h + 1) * P, lo:hi]
                            .rearrange("(o p) t -> p o t", p=P))
                return xs, pieces

            def chain_x_pair(kp, xs, pieces):
                for h in range(2):
                    for lo, hi in pieces:
                        # all x affines on ScalarE: prologue DVE (clips,
                        # w chains) is the gating engine in the trace
                        quant_chain(xq_tiles[kp][:, h, lo:hi],
                                    xs[:, h, lo:hi], s_x,
                                    aff_scalar=True)

            def quant_x_pair(kp, fine=False):
                xs, pieces = issue_x_pair(kp, fine)
                chain_x_pair(kp, xs, pieces)

            def w_fine_sub(wq, mt, a, b, dma_engine=None):
                # prologue w chains: the fp32 affine (slow 1x op) rides
                # ScalarE, which is light in the prologue; only the 4x
                # fp16 clip/sub stay on the loaded DVE
                ws = wsf_pool.tile([P, b - a, P], dt.float32, name="wsf")
                (dma_engine or nc.sync).dma_start(
                    ws[:],
                    wt[mt, a * P:b * P, :]
                    .rearrange("(o p) f -> p o f", p=P))
                quant_chain(wq[:, a:b, :], ws[:], s_w, aff_scalar=True)

            def prep_w_fine(mt, subs):
                # prologue m-tiles: sub-chunk chains (all on DVE) write
                # disjoint k-slices of one wq tile so the first matmuls
                # only wait on sub 0.
                wq = wq_pool.tile([P, KT, P], dt.float16, name="wq", tag="wq")
                for a, b in subs:
                    w_fine_sub(wq, mt, a, b)
                return wq

            def prep_w_big(mt):
                # steady state: 2 half DMAs + 2 half chains on DVE
                wq = wq_pool.tile([P, KT, P], dt.float16, name="wq", tag="wq")
                ws = wsb_pool.tile([P, KT, P], dt.float32, name="wsb")
                h = KT // 2
                for q in range(2):
                    nc.sync.dma_start(
                        ws[:, q * h:(q + 1) * h, :],
                        wt[mt, q * h * P:(q + 1) * h * P, :]
                        .rearrange("(o p) f -> p o f", p=P))
                    quant_chain(wq[:, q * h:(q + 1) * h, :],
                                ws[:, q * h:(q + 1) * h, :], s_w,
                                aff_scalar=False)
                return wq

            def alloc_ps():
                return [psum_pool.tile([P, TF], dt.float32, name=f"ps{i}")
                        for i in range(NTF)]

            def mm_kt(pss, wq, kt):
                for tf in range(NTF):
                    nc.tensor.matmul(
                        pss[tf][:],
                        wq[:, kt, :],
                        xq_tiles[kt // 2][:, kt % 2,
                                          tf * TF:(tf + 1) * TF],
                        start=(kt == 0), stop=(kt == KT - 1),
                    )

            def mm_chunk(pss, wq, c):
                for kt in range(c * 2 * PPC, (c + 1) * 2 * PPC):
                    mm_kt(pss, wq, kt)

            def store(mt, pss, last=False):
                # dequant on ScalarE straight out of PSUM; the out DMA is
                # issued from ScalarE right behind it (wait already met).
                # For the very last tile the final bank is split in half
                # with the DMA issues on two engines to shorten the tail.
                for tf in range(NTF):
                    outt = out_pool.tile([P, TF], dt.float16, name="outt")
                    nc.scalar.activation(
                        outt[:], pss[tf][:],
                        AF.Identity, bias=bias_sb[:, mt:mt + 1],
                        scale=inv_s)
                    if last and tf == NTF - 1:
                        h = TF // 2
                        nc.sync.dma_start(
                            out[mt * P:(mt + 1) * P,
                                tf * TF + h:(tf + 1) * TF],
                            outt[:, h:TF])
                        nc.scalar.dma_start(
                            out[mt * P:(mt + 1) * P,
                                tf * TF:tf * TF + h],
                            outt[:, 0:h])
                    else:
                        nc.scalar.dma_start(
                            out[mt * P:(mt + 1) * P,
                                tf * TF:(tf + 1) * TF],
                            outt[:])

            # ---- prologue --------------------------------------------
            # The prologue is DMA-bandwidth-bound (x's 16.8MB plus the
            # early w tiles saturate all 16 queues), so x-pair DMAs are
            # issued ahead of weight DMAs, w sub-chunks arrive just in
            # time for the wave that consumes them, and junk matmuls
            # plug the unavoidable arrival gaps so the PE clock never
            # drops out of its ramped pstate.
            xs0 = xs_pool.tile([P, 2, TPC], dt.float32, name="xs")
            nc.sync.dma_start(
                xs0[:, 0, 0:TF],
                xt[0:P, 0:TF].rearrange("(o p) t -> p o t", p=P))
            nc.vector.memset(junk_sb, 0.0)
            nc.sync.dma_start(
                xs0[:, 0, TF:TPC],
                xt[0:P, TF:TPC].rearrange("(o p) t -> p o t", p=P))
            nc.sync.dma_start(
                xs0[:, 1, :],
                xt[P:2 * P, :].rearrange("(o p) t -> p o t", p=P))
            bias_sb = cst_pool.tile([P, MT], dt.float32, name="bias_sb")
            nc.scalar.dma_start(bias_sb[:], bias[:, :])

            wqs = {}
            pro_ps = {}
            PRO = 4
            # psum slot rotation (bufs=4): allocate 0 first and the junk
            # bank (shared with m-tile 3) second, so steady-state m-tile 4
            # reuses m-tile 0's long-retired banks, not m-tile 3's.
            pro_ps[0] = alloc_ps()
            pro_ps[PRO - 1] = alloc_ps()
            pro_ps[1] = alloc_ps()
            pro_ps[2] = alloc_ps()
            junk_mms(JUNK_PRE, pro_ps[PRO - 1][0])

            # m-tile 0's first w sub-chunk: DMA issued from ScalarE (its
            # first instruction) so it's in flight before any engine blocks
            wq0 = wq_pool.tile([P, KT, P], dt.float16, name="wq", tag="wq")
            wqs[0] = wq0
            w_fine_sub(wq0, 0, 0, 4, dma_engine=nc.scalar)
            quant_chain(xq_tiles[0][:, 0, 0:TF], xs0[:, 0, 0:TF], s_x, True)
            quant_chain(xq_tiles[0][:, 0, TF:TPC], xs0[:, 0, TF:TPC],
                        s_x, False)
            quant_chain(xq_tiles[0][:, 1, :], xs0[:, 1, :], s_x, True)
            quant_x_pair(1, fine=True)
            for a, b in [(4, 8), (8, 16), (16, 24), (24, 32)]:
                w_fine_sub(wq0, 0, a, b)

            # wave 0: m-tile 0 eats chunk 0 pair-by-pair, junk plugs the
            # arrival gaps (and keeps the PE clock ramped).  x-pair DMAs
            # are issued a full wave ahead (pend holds landed-but-
            # unchained pairs) so arrivals never gate the consuming wave.
            JW = [4, 3, 2, 1]
            pend = {}
            # issue priority: this wave's pairs, next m-tile's first w
            # sub (gates wave 1's first matmuls), then next wave's pairs
            for kp in (2, 3):
                pend[kp] = issue_x_pair(kp, fine=True)
            wq1 = wq_pool.tile([P, KT, P], dt.float16, name="wq", tag="wq")
            wqs[1] = wq1
            w_fine_sub(wq1, 1, 0, 8)
            for kp in range(PPC, 2 * PPC):
                pend[kp] = issue_x_pair(kp, False)
            for kp in range(PPC):
                if kp >= 2:
                    xs_, pieces_ = pend.pop(kp)
                    chain_x_pair(kp, xs_, pieces_)
                mm_kt(pro_ps[0], wqs[0], 2 * kp)
                mm_kt(pro_ps[0], wqs[0], 2 * kp + 1)
                junk_mms(JW[0], pro_ps[PRO - 1][0])
            for a, b in [(8, 16), (16, 24), (24, 32)]:
                w_fine_sub(wq1, 1, a, b)

            # waves 1..3: the wave's x pairs first (SP-issue priority and
            # quant), then the next m-tile's whole weight prep (a full
            # wave of slack before first use), then the older m-tiles'
            # ready chunks and m-tile 0's fresh chunk.  Junk shares
            # m-tile 3's psum bank: before m-tile 3's accumulation opens
            # it resets the bank (start=True, harmless), afterwards it
            # rides along as zero-accumulation (start=False, adds 0.0).
            mt3_live = False
            for c in range(1, XCH):
                # next prologue m-tile's first w sub leads the issue
                # order (it gates the next wave's first matmul chunk)
                if c + 1 < PRO:
                    wqn = wq_pool.tile([P, KT, P], dt.float16, name="wq",
                                       tag="wq")
                    wqs[c + 1] = wqn
                    w_fine_sub(wqn, c + 1, 0, 8)
                if c + 1 < XCH:
                    for kp in range((c + 1) * PPC, (c + 2) * PPC):
                        pend[kp] = issue_x_pair(kp, False)
                for kp in range(c * PPC, (c + 1) * PPC):
                    xs, pieces = pend.pop(kp)
                    chain_x_pair(kp, xs, pieces)
                if c + 1 < PRO:
                    for a, b in [(8, 16), (16, 24), (24, 32)]:
                        w_fine_sub(wqs[c + 1], c + 1, a, b)
                for mt in range(1, PRO):
                    cc = c - mt
                    if 0 <= cc < XCH:
                        mm_chunk(pro_ps[mt], wqs[mt], cc)
                        if mt == PRO - 1:
                            mt3_live = True
                        junk_mms(JW[c], pro_ps[PRO - 1][0], live=mt3_live)
                mm_chunk(pro_ps[0], wqs[0], c)
                junk_mms(JW[c], pro_ps[PRO - 1][0], live=mt3_live)

            # closing waves: finish and store prologue m-tiles; the first
            # steady-state w preps go right behind the stores that retire
            # the wq/psum slots they will reuse (issuing them earlier would
            # deadlock the in-order DVE queue on the pool-slot WAR wait)
            for mt in range(PRO):
                for cc in range(XCH - mt, XCH):
                    mm_chunk(pro_ps[mt], wqs[mt], cc)
                store(mt, pro_ps[mt])
                if mt < 2:
                    wqs[PRO + mt] = prep_w_big(PRO + mt)

            # ---- steady-state m-loop, pipelined two m-tiles ahead ----
            for mt in range(PRO, MT):
                wq = wqs.pop(mt)
                if mt + 2 < MT:
                    wqs[mt + 2] = prep_w_big(mt + 2)
                pss = alloc_ps()
                if mt == MT - 1:
                    # last m-tile: run the two 512-token banks serially
                    # so bank0's dequant+store hides under bank1's
                    # matmuls and only bank1's store is left in the tail
                    for tf in range(NTF):
                        for kt in range(KT):
                            nc.tensor.matmul(
                                pss[tf][:], wq[:, kt, :],
                                xq_tiles[kt // 2][:, kt % 2,
                                                  tf * TF:(tf + 1) * TF],
                                start=(kt == 0), stop=(kt == KT - 1))
                        outt = out_pool.tile([P, TF], dt.float16,
                                             name="outt")
                        nc.scalar.activation(
                            outt[:], pss[tf][:], AF.Identity,
                            bias=bias_sb[:, mt:mt + 1], scale=inv_s)
                        h = TF // 2
                        if tf == NTF - 1:
                            nc.sync.dma_start(
                                out[mt * P:(mt + 1) * P,
                                    tf * TF + h:(tf + 1) * TF],
                                outt[:, h:TF])
                            nc.scalar.dma_start(
                                out[mt * P:(mt + 1) * P,
                                    tf * TF:tf * TF + h],
                                outt[:, 0:h])
                        else:
                            nc.scalar.dma_start(
                                out[mt * P:(mt + 1) * P,
                                    tf * TF:(tf + 1) * TF],
                                outt[:])
                else:
                    for kt in range(KT):
                        mm_kt(pss, wq, kt)
                    store(mt, pss)

    nc.compile()
    return nc

def _prep(x, weight, bias, amax_x, amax_w):
    ax = np.float32(np.asarray(amax_x, dtype=np.float32).reshape(-1)[0])
    aw = np.float32(np.asarray(amax_w, dtype=np.float32).reshape(-1)[0])
    s_x = np.float32(127.0) / ax
    s_w = np.float32(127.0) / aw
    inv_s = np.float32(1.0) / (s_x * s_w)

    x = np.asarray(x, dtype=np.float32)
    weight = np.asarray(weight, dtype=np.float32)
    bias = np.asarray(bias, dtype=np.float32)

    xT = np.ascontiguousarray(x.T)  # [K, N]
    # [MT, K, 128]: per m-tile a contiguous k-major block of W^T
    wt3 = np.ascontiguousarray(weight.reshape(MT, P, K).transpose(0, 2, 1))
    in_maps = [
        {
            "xt": np.ascontiguousarray(xT[:, c * TPC:(c + 1) * TPC]),
            "wt": wt3,
            "bias": np.ascontiguousarray(bias.reshape(MT, P).T),
        }
        for c in range(N_CORES)
    ]
    return float(s_x), float(s_w), float(inv_s), in_maps


def _spot_check(full, x, weight, bias, amax_x, amax_w, n=8):
    """Cheap host-side validation of a few output elements against the exact
    quantized-GEMM reference; catches transient device faults (observed as
    both exec errors and corrupted outputs on this fleet)."""
    rng = np.random.default_rng(0)
    ii = rng.integers(0, x.shape[0], size=n)
    jj = rng.integers(0, weight.shape[0], size=n)
    ax = np.float32(np.asarray(amax_x, np.float32).reshape(-1)[0])
    aw = np.float32(np.asarray(amax_w, np.float32).reshape(-1)[0])
    s_x = np.float32(127.0) / ax
    s_w = np.float32(127.0) / aw
    for i, j in zip(ii, jj):
        qx = np.clip(np.round(x[i].astype(np.float32) * s_x), -127, 127)
        qw = np.clip(np.round(weight[j].astype(np.float32) * s_w), -127, 127)
        exp = float(qx @ qw) / float(s_x * s_w) + float(bias[j])
        if abs(float(full[i, j]) - exp) > 1e-2 * max(1.0, abs(exp)):
            return False
    return True


def run(x, weight, bias, amax_x, amax_w, trace: bool = False):
    from concourse.bass_utils import run_bass_kernel_spmd

    s_x, s_w, inv_s, in_maps = _prep(x, weight, bias, amax_x, amax_w)
    nc = build(s_x, s_w, inv_s)
    full = None
    res = None
    err = None
    for attempt in range(3):
        try:
            res = run_bass_kernel_spmd(nc, in_maps,
                                       core_ids=list(range(N_CORES)),
                                       trace=trace)
            shards = [res.results[c]["out"] for c in range(N_CORES)]
            full = np.concatenate([s.T for s in shards],
                                  axis=0).astype(np.float32)
            if _spot_check(full, x, weight, bias, amax_x, amax_w):
                return full, res
        except Exception as e:  # transient NRT exec faults: retry
            err = e
    if full is not None:
        return full, res
    raise err


def kernel(x, weight, bias, amax_x, amax_w):
    full, _ = run(x, weight, bias, amax_x, amax_w, trace=False)
    return full



# revision 50
# speedup vs baseline: 1.0311x; 1.0311x over previous
"""AdaPT quantized linear (int8-exact via fp16 matmul) on 8 TRN2 NeuronCores.

Reference computes:
    qx = clip(round(x * 127/amax_x), -127, 127)        [N, K] int8
    qw = clip(round(w * 127/amax_w), -127, 127)        [M, K] int8
    out = (qx @ qw.T) / ((127/amax_x)*(127/amax_w)) + bias

Strategy: data-parallel over the 8192-token dim (1024 tokens/core), full
weight on every core, no collectives.  All int8 values are exactly
representable in fp16, the PE multiplies fp16 exactly (products < 2^14)
and accumulates in fp32 (partial sums << 2^24), so the fp16 matmul
reproduces the int8 systolic GEMM bit-exactly.

Rounding trick: fp16 has unit spacing on [1024, 2048), so converting
(x*scale + 1536) from fp32 to fp16 rounds the fractional part to the
nearest integer with ties-to-even -- exactly jnp.round.  The quant chain
is then: affine convert to fp16 (ScalarE Copy with immediate bias, or DVE
mult+add dual-op), then fp16 min/max clip and subtract on DVE, which runs
16-bit SBUF-to-SBUF ops at 4x rate.

Engine plan (per core):
  - PE: 2048 accumulating fp16 matmuls (32 m-tiles x 32 k-tiles x 2
    banks of 512 tokens) -- the 437us roofline everything else hides
    under.  A short junk-matmul burst covers the pstate ramp; junk is
    also interleaved into the first wave so the PE never idles (idle
    resets the clock ramp).  (fp8 DoubleRow was measured at 2x per
    k-tile on HW, so any exact 3-product int8 split is 1.5x slower
    than this single fp16 pass -- not viable.)
  - ScalarE: x affine converts (half the pairs), prologue w affine
    converts (fp32 1x ops moved off the loaded DVE), PSUM dequant+bias,
    and the output-store DMA issues.
  - DVE: everything fp16 (clips/subtracts at 4x), steady-state w
    affines, x affine for the other half of the pairs.
  - SP(sync): input DMA issues only.  dma_start blocks its issuing
    engine in program order, so weight DMAs are issued 2 m-tiles ahead
    and never behind dependent work.

The prologue is DMA-bandwidth-bound, not engine-bound: x's 16.8MB plus
the first ~6 weight tiles saturate the DMA for the first ~70us, which
caps how early the 4 PSUM-open m-tiles can retire (structural deficit
~15us).  The wave schedule (x-priority issue order, half-token x
pieces in the first waves, fine w sub-chunks with a full wave of
slack) sits essentially at that bound.  bias is pre-transposed to
[P, MT] on the host: the naive [M]->(p,o) gather DMA stalled its
issuing queue ~11us right when the x-pair issues mattered.  The last
m-tile runs its two banks serially so only one bank's dequant+store
remains after the final matmul (~2us tail).

Output is stored as fp16 ([M, tokens/core] transposed per core, so the
bias rides the ScalarE per-partition bias port) and upcast on the host;
the ~3e-4 relative rounding is far inside the 2e-2 gate.
"""

import sys

import numpy as np

sys.path.insert(0, "/opt/trn_rl_repo")

N, K, M = 8192, 4096, 4096
N_CORES = 8
TPC = N // N_CORES  # tokens per core
P = 128
KT = K // P   # 32 k-tiles
MT = M // P   # 32 m-tiles
TF = 512      # matmul moving free dim (one PSUM bank of fp32)
NTF = TPC // TF
XCH = 4       # x chunks (waves); 4 pairs (8 k-tiles) each
NPAIR = KT // 2
PPC = NPAIR // XCH
MAGIC = 1536.0   # fp16 unit-spacing window base: round via fp32->fp16 convert
MAXV = 127.0
HI = MAGIC + MAXV
LO = MAGIC - MAXV
JUNK_PRE = 12    # PE pstate-ramp warmup matmuls before first real work
JUNK_WAVE = 2    # junk interleaved between wave-0 pair groups


def build(s_x: float, s_w: float, inv_s: float):
    import concourse.mybir as mybir
    import concourse.tile as tile
    from concourse import bacc

    dt = mybir.dt
    AF = mybir.ActivationFunctionType
    OP = mybir.AluOpType

    nc = bacc.Bacc("TRN2", target_bir_lowering=False, debug=False,
                   num_devices=N_CORES)

    xt = nc.declare_dram_parameter("xt", [K, TPC], dt.float32, isOutput=False)
    wt = nc.declare_dram_parameter("wt", [MT, K, P], dt.float32, isOutput=False)
    # bias pre-transposed to [P, MT] on the host: the [M]->(p,o) gather
    # was a 4096x4B strided DMA that blocked the Sync queue ~11us right
    # when the x-pair DMA issues mattered most
    bias = nc.declare_dram_parameter("bias", [P, MT], dt.float32,
                                     isOutput=False)
    out = nc.declare_dram_parameter("out", [M, TPC], dt.float16, isOutput=True)

    with tile.TileContext(nc) as tc:
        with (
            tc.tile_pool(name="xq", bufs=1) as xq_pool,
            tc.tile_pool(name="xs", bufs=6) as xs_pool,
            tc.tile_pool(name="wsf", bufs=4) as wsf_pool,
            tc.tile_pool(name="wsb", bufs=2) as wsb_pool,
            tc.tile_pool(name="wq", bufs=4) as wq_pool,
            tc.tile_pool(name="cst", bufs=1) as cst_pool,
            tc.tile_pool(name="outp", bufs=6) as out_pool,
            tc.tile_pool(name="ps", bufs=4, space="PSUM") as psum_pool,
            tc.tile_pool(name="junk", bufs=1) as junk_pool,
        ):
            # ---- bookkeeping helpers ---------------------------------
            # Junk buffer lives OUTSIDE tile tracking: the first warmup
            # matmuls read garbage (their start=True target is reset by
            # the first real matmul anyway) without waiting on the memset.
            # The memset zeroes it long before any start=False junk runs,
            # so post-start junk accumulates exactly 0 into live banks.
            junk_t = nc.alloc_sbuf_tensor("junk_sb", [P, TF], dt.float16)
            junk_sb = junk_t.ap()

            def junk_mms(n, tgt, live=False):
                # live=True: the target bank's accumulation group is open;
                # zero-junk rides along with start=False (adds 0.0).
                for _ in range(n):
                    nc.tensor.matmul(tgt[:], junk_sb[:, :P], junk_sb[:],
                                     start=not live, stop=False,
                                     skip_group_check=True)

            xq_tiles = [
                xq_pool.tile([P, 2, TPC], dt.float16, name=f"xq{p}",
                             tag=f"xq{p}")
                for p in range(NPAIR)
            ]

            def quant_chain(dst, src, scale, aff_scalar):
                # dst fp16 = clip(round(src*scale), -127, 127) + 1536,
                # recentred; the affine's fp32->fp16 convert is the round.
                if aff_scalar:
                    nc.scalar.activation(dst, src, AF.Copy,
                                         bias=MAGIC, scale=scale)
                else:
                    nc.vector.tensor_scalar(dst, src, scale, MAGIC,
                                            OP.mult, OP.add)
                nc.vector.tensor_scalar(dst, dst, HI, LO, OP.min, OP.max)
                nc.vector.tensor_scalar(dst, dst, MAGIC, None, OP.subtract)

            def issue_x_pair(kp, fine=False):
                # per-k-tile DMAs (0.5MB arrival quantum).  fine=True
                # (DMA-bound early waves): split each k-tile into token
                # halves so the bank-0 matmuls can start ~1.5us before
                # the k-tile's second half lands.
                kt0 = 2 * kp
                xs = xs_pool.tile([P, 2, TPC], dt.float32, name="xs")
                pieces = ((0, TF), (TF, TPC)) if fine else ((0, TPC),)
                for h in range(2):
                    for lo, hi in pieces:
                        nc.sync.dma_start(
                            xs[:, h, lo:hi],
                            xt[(kt0 + h) * P:(kt0 + h + 1) * P, lo:hi]
                            .rearrange("(o p) t -> p o t", p=P))
                return xs, pieces

            def chain_x_pair(kp, xs, pieces):
                for h in range(2):
                    for lo, hi in pieces:
                        # all x affines on ScalarE: prologue DVE (clips,
                        # w chains) is the gating engine in the trace
                        quant_chain(xq_tiles[kp][:, h, lo:hi],
                                    xs[:, h, lo:hi], s_x,
                                    aff_scalar=True)

            def quant_x_pair(kp, fine=False):
                xs, pieces = issue_x_pair(kp, fine)
                chain_x_pair(kp, xs, pieces)

            def w_fine_sub(wq, mt, a, b, dma_engine=None):
                # prologue w chains: the fp32 affine (slow 1x op) rides
                # ScalarE, which is light in the prologue; only the 4x
                # fp16 clip/sub stay on the loaded DVE
                ws = wsf_pool.tile([P, b - a, P], dt.float32, name="wsf")
                (dma_engine or nc.sync).dma_start(
                    ws[:],
                    wt[mt, a * P:b * P, :]
                    .rearrange("(o p) f -> p o f", p=P))
                quant_chain(wq[:, a:b, :], ws[:], s_w, aff_scalar=True)

            def prep_w_fine(mt, subs):
                # prologue m-tiles: sub-chunk chains (all on DVE) write
                # disjoint k-slices of one wq tile so the first matmuls
                # only wait on sub 0.
                wq = wq_pool.tile([P, KT, P], dt.float16, name="wq", tag="wq")
                for a, b in subs:
                    w_fine_sub(wq, mt, a, b)
                return wq

            def prep_w_big(mt):
                # steady state: 2 half DMAs + 2 half chains on DVE
                wq = wq_pool.tile([P, KT, P], dt.float16, name="wq", tag="wq")
                ws = wsb_pool.tile([P, KT, P], dt.float32, name="wsb")
                h = KT // 2
                for q in range(2):
                    nc.sync.dma_start(
                        ws[:, q * h:(q + 1) * h, :],
                        wt[mt, q * h * P:(q + 1) * h * P, :]
                        .rearrange("(o p) f -> p o f", p=P))
                    quant_chain(wq[:, q * h:(q + 1) * h, :],
                                ws[:, q * h:(q + 1) * h, :], s_w,
                                aff_scalar=False)
                return wq

            def alloc_ps():
                return [psum_pool.tile([P, TF], dt.float32, name=f"ps{i}")
                        for i in range(NTF)]

            def mm_kt(pss, wq, kt):
                for tf in range(NTF):
                    nc.tensor.matmul(
                        pss[tf][:],
                        wq[:, kt, :],
                        xq_tiles[kt // 2][:, kt % 2,
                                          tf * TF:(tf + 1) * TF],
                        start=(kt == 0), stop=(kt == KT - 1),
                    )

            def mm_chunk(pss, wq, c):
                for kt in range(c * 2 * PPC, (c + 1) * 2 * PPC):
                    mm_kt(pss, wq, kt)

            def store(mt, pss, last=False):
                # dequant on ScalarE straight out of PSUM; the out DMA is
                # issued from ScalarE right behind it (wait already met).
                # For the very last tile the final bank is split in half
                # with the DMA issues on two engines to shorten the tail.
                for tf in range(NTF):
                    outt = out_pool.tile([P, TF], dt.float16, name="outt")
                    nc.scalar.activation(
                        outt[:], pss[tf][:],
                        AF.Identity, bias=bias_sb[:, mt:mt + 1],
                        scale=inv_s)
                    if last and tf == NTF - 1:
                        h = TF // 2
                        nc.sync.dma_start(
                            out[mt * P:(mt + 1) * P,
                                tf * TF + h:(tf + 1) * TF],
                            outt[:, h:TF])
                        nc.scalar.dma_start(
                            out[mt * P:(mt + 1) * P,
                                tf * TF:tf * TF + h],
                            outt[:, 0:h])
                    else:
                        nc.scalar.dma_start(
                            out[mt * P:(mt + 1) * P,
                                tf * TF:(tf + 1) * TF],
                            outt[:])

            # ---- prologue --------------------------------------------
            # The prologue is DMA-bandwidth-bound (x's 16.8MB plus the
            # early w tiles saturate all 16 queues), so x-pair DMAs are
            # issued ahead of weight DMAs, w sub-chunks arrive just in
            # time for the wave that consumes them, and junk matmuls
            # plug the unavoidable arrival gaps so the PE clock never
            # drops out of its ramped pstate.
            xs0 = xs_pool.tile([P, 2, TPC], dt.float32, name="xs")
            nc.sync.dma_start(
                xs0[:, 0, 0:TF],
                xt[0:P, 0:TF].rearrange("(o p) t -> p o t", p=P))
            nc.vector.memset(junk_sb, 0.0)
            nc.sync.dma_start(
                xs0[:, 0, TF:TPC],
                xt[0:P, TF:TPC].rearrange("(o p) t -> p o t", p=P))
            nc.sync.dma_start(
                xs0[:, 1, :],
                xt[P:2 * P, :].rearrange("(o p) t -> p o t", p=P))
            bias_sb = cst_pool.tile([P, MT], dt.float32, name="bias_sb")
            nc.scalar.dma_start(bias_sb[:], bias[:, :])

            wqs = {}
            pro_ps = {}
            PRO = 4
            # psum slot rotation (bufs=4): allocate 0 first and the junk
            # bank (shared with m-tile 3) second, so steady-state m-tile 4
            # reuses m-tile 0's long-retired banks, not m-tile 3's.
            pro_ps[0] = alloc_ps()
            pro_ps[PRO - 1] = alloc_ps()
            pro_ps[1] = alloc_ps()
            pro_ps[2] = alloc_ps()
            junk_mms(JUNK_PRE, pro_ps[PRO - 1][0])

            # m-tile 0's first w sub-chunk: DMA issued from ScalarE (its
            # first instruction) so it's in flight before any engine blocks
            wq0 = wq_pool.tile([P, KT, P], dt.float16, name="wq", tag="wq")
            wqs[0] = wq0
            w_fine_sub(wq0, 0, 0, 4, dma_engine=nc.scalar)
            quant_chain(xq_tiles[0][:, 0, 0:TF], xs0[:, 0, 0:TF], s_x, True)
            quant_chain(xq_tiles[0][:, 0, TF:TPC], xs0[:, 0, TF:TPC],
                        s_x, False)
            quant_chain(xq_tiles[0][:, 1, :], xs0[:, 1, :], s_x, True)
            quant_x_pair(1, fine=True)
            for a, b in [(4, 8), (8, 16), (16, 24), (24, 32)]:
                w_fine_sub(wq0, 0, a, b)

            # wave 0: m-tile 0 eats chunk 0 pair-by-pair, junk plugs the
            # arrival gaps (and keeps the PE clock ramped).  x-pair DMAs
            # are issued a full wave ahead (pend holds landed-but-
            # unchained pairs) so arrivals never gate the consuming wave.
            JW = [4, 3, 2, 1]
            pend = {}
            # issue priority: this wave's pairs, next m-tile's first w
            # sub (gates wave 1's first matmuls), then next wave's pairs
            for kp in (2, 3):
                pend[kp] = issue_x_pair(kp, fine=True)
            wq1 = wq_pool.tile([P, KT, P], dt.float16, name="wq", tag="wq")
            wqs[1] = wq1
            w_fine_sub(wq1, 1, 0, 8)
            for kp in range(PPC, 2 * PPC):
                pend[kp] = issue_x_pair(kp, False)
            for kp in range(PPC):
                if kp >= 2:
                    xs_, pieces_ = pend.pop(kp)
                    chain_x_pair(kp, xs_, pieces_)
                mm_kt(pro_ps[0], wqs[0], 2 * kp)
                mm_kt(pro_ps[0], wqs[0], 2 * kp + 1)
                junk_mms(JW[0], pro_ps[PRO - 1][0])
            for a, b in [(8, 16), (16, 24), (24, 32)]:
                w_fine_sub(wq1, 1, a, b)

            # waves 1..3: the wave's x pairs first (SP-issue priority and
            # quant), then the next m-tile's whole weight prep (a full
            # wave of slack before first use), then the older m-tiles'
            # ready chunks and m-tile 0's fresh chunk.  Junk shares
            # m-tile 3's psum bank: before m-tile 3's accumulation opens
            # it resets the bank (start=True, harmless), afterwards it
            # rides along as zero-accumulation (start=False, adds 0.0).
            mt3_live = False
            for c in range(1, XCH):
                # next prologue m-tile's first w sub leads the issue
                # order (it gates the next wave's first matmul chunk)
                if c + 1 < PRO:
                    wqn = wq_pool.tile([P, KT, P], dt.float16, name="wq",
                                       tag="wq")
                    wqs[c + 1] = wqn
                    w_fine_sub(wqn, c + 1, 0, 8)
                if c + 1 < XCH:
                    for kp in range((c + 1) * PPC, (c + 2) * PPC):
                        pend[kp] = issue_x_pair(kp, False)
                for kp in range(c * PPC, (c + 1) * PPC):
                    xs, pieces = pend.pop(kp)
                    chain_x_pair(kp, xs, pieces)
                if c + 1 < PRO:
                    for a, b in [(8, 16), (16, 24), (24, 32)]:
                        w_fine_sub(wqs[c + 1], c + 1, a, b)
                for mt in range(1, PRO):
                    cc = c - mt
                    if 0 <= cc < XCH:
                        mm_chunk(pro_ps[mt], wqs[mt], cc)
                        if mt == PRO - 1:
                            mt3_live = True
                        junk_mms(JW[c], pro_ps[PRO - 1][0], live=mt3_live)
                mm_chunk(pro_ps[0], wqs[0], c)
                junk_mms(JW[c], pro_ps[PRO - 1][0], live=mt3_live)

            # closing waves: finish and store prologue m-tiles; the first
            # steady-state w preps go right behind the stores that retire
            # the wq/psum slots they will reuse (issuing them earlier would
            # deadlock the in-order DVE queue on the pool-slot WAR wait)
            for mt in range(PRO):
                for cc in range(XCH - mt, XCH):
                    mm_chunk(pro_ps[mt], wqs[mt], cc)
                store(mt, pro_ps[mt])
                if mt < 2:
                    wqs[PRO + mt] = prep_w_big(PRO + mt)

            # ---- steady-state m-loop, pipelined two m-tiles ahead ----
            for mt in range(PRO, MT):
                wq = wqs.pop(mt)
                if mt + 2 < MT:
                    wqs[mt + 2] = prep_w_big(mt + 2)
                pss = alloc_ps()
                if mt == MT - 1:
                    # last m-tile: run the two 512-token banks serially
                    # so bank0's dequant+store hides under bank1's
                    # matmuls and only bank1's store is left in the tail
                    for tf in range(NTF):
                        for kt in range(KT):
                            nc.tensor.matmul(
                                pss[tf][:], wq[:, kt, :],
                                xq_tiles[kt // 2][:, kt % 2,
                                                  tf * TF:(tf + 1) * TF],
                                start=(kt == 0), stop=(kt == KT - 1))
                        outt = out_pool.tile([P, TF], dt.float16,
                                             name="outt")
                        nc.scalar.activation(
                            outt[:], pss[tf][:], AF.Identity,
                            bias=bias_sb[:, mt:mt + 1], scale=inv_s)
                        h = TF // 2
                        if tf == NTF - 1:
                            nc.sync.dma_start(
                                out[mt * P:(mt + 1) * P,
                                    tf * TF + h:(tf + 1) * TF],
                                outt[:, h:TF])
                            nc.scalar.dma_start(
                                out[mt * P:(mt + 1) * P,
                                    tf * TF:tf * TF + h],
                                outt[:, 0:h])
                        else:
                            nc.scalar.dma_start(
                                out[mt * P:(mt + 1) * P,
                                    tf * TF:(tf + 1) * TF],
                                outt[:])
                else:
                    for kt in range(KT):
                        mm_kt(pss, wq, kt)
                    store(mt, pss)

    nc.compile()
    return nc

def _prep(x, weight, bias, amax_x, amax_w):
    ax = np.float32(np.asarray(amax_x, dtype=np.float32).reshape(-1)[0])
    aw = np.float32(np.asarray(amax_w, dtype=np.float32).reshape(-1)[0])
    s_x = np.float32(127.0) / ax
    s_w = np.float32(127.0) / aw
    inv_s = np.float32(1.0) / (s_x * s_w)

    x = np.asarray(x, dtype=np.float32)
    weight = np.asarray(weight, dtype=np.float32)
    bias = np.asarray(bias, dtype=np.float32)

    xT = np.ascontiguousarray(x.T)  # [K, N]
    # [MT, K, 128]: per m-tile a contiguous k-major block of W^T
    wt3 = np.ascontiguousarray(weight.reshape(MT, P, K).transpose(0, 2, 1))
    in_maps = [
        {
            "xt": np.ascontiguousarray(xT[:, c * TPC:(c + 1) * TPC]),
            "wt": wt3,
            "bias": np.ascontiguousarray(bias.reshape(MT, P).T),
        }
        for c in range(N_CORES)
    ]
    return float(s_x), float(s_w), float(inv_s), in_maps


def _spot_check(full, x, weight, bias, amax_x, amax_w, n=8):
    """Cheap host-side validation of a few output elements against the exact
    quantized-GEMM reference; catches transient device faults (observed as
    both exec errors and corrupted outputs on this fleet)."""
    rng = np.random.default_rng(0)
    ii = rng.integers(0, x.shape[0], size=n)
    jj = rng.integers(0, weight.shape[0], size=n)
    ax = np.float32(np.asarray(amax_x, np.float32).reshape(-1)[0])
    aw = np.float32(np.asarray(amax_w, np.float32).reshape(-1)[0])
    s_x = np.float32(127.0) / ax
    s_w = np.float32(127.0) / aw
    for i, j in zip(ii, jj):
        qx = np.clip(np.round(x[i].astype(np.float32) * s_x), -127, 127)
        qw = np.clip(np.round(weight[j].astype(np.float32) * s_w), -127, 127)
        exp = float(qx @ qw) / float(s_x * s_w) + float(bias[j])
        if abs(float(full[i, j]) - exp) > 1e-2 * max(1.0, abs(exp)):
            return False
    return True


def run(x, weight, bias, amax_x, amax_w, trace: bool = False):
    from concourse.bass_utils import run_bass_kernel_spmd

    s_x, s_w, inv_s, in_maps = _prep(x, weight, bias, amax_x, amax_w)
    nc = build(s_x, s_w, inv_s)
    full = None
    res = None
    err = None
    for attempt in range(3):
        try:
            res = run_bass_kernel_spmd(nc, in_maps,
                                       core_ids=list(range(N_CORES)),
                                       trace=trace)
            shards = [res.results[c]["out"] for c in range(N_CORES)]
            full = np.concatenate([s.T for s in shards],
                                  axis=0).astype(np.float32)
            if _spot_check(full, x, weight, bias, amax_x, amax_w):
                return full, res
        except Exception as e:  # transient NRT exec faults: retry
            err = e
    if full is not None:
        return full, res
    raise err


def kernel(x, weight, bias, amax_x, amax_w):
    full, _ = run(x, weight, bias, amax_x, amax_w, trace=False)
    return full

